# revision 25
# baseline (speedup 1.0000x reference)
"""Trainium2 Bass kernel for nn_Decoder_16054587752897.

Decoder block: banded additive (Bahdanau) attention + LN + FFN + LN +
3x (nearest-upsample-2x + conv1d k=7 + relu) + conv1d k=11 + sigmoid.

Sharding: pure data parallel - batch N=8, one batch element per NeuronCore.
All parameters replicated; each core computes its (96, 512) -> (1, 4096) slice.

Device-side structure (per core):
 - q/k projections via PE col-tiled matmuls into [(chunk,d)=128, i=128] layout.
 - Banded attention: never materializes the (L, L, d) intermediate. For each
   band offset o in [0,64): tanh(Q4 + K4pad[:, o:o+128]) via one AP-broadcast
   DVE add + ACT tanh per 16-offset group, then per-o matmuls with the tanh
   tile as lhsT and a block-diagonal Wa as rhs, accumulating band logits
   E[i_local, (chunk, o)] into a single PSUM bank.
 - Band softmax without max subtraction (|e| <= sum|Wa| ~ 1.3 so exp is safe;
   the reference's row max only affects the +1e-6 eps term, ~1e-7 relative).
 - Normalized weights scattered to a DRAM scratch in dense banded row layout
   (contiguous innermost on both sides), read back as dense [i, j'] tiles,
   PE-transposed, and contracted against PE-transposed X windows -> v^T.
 - LN stats via ones-vector matmuls (partition reduction) + rank-1 PE
   broadcast; rstd = Newton-reciprocal(ACT sqrt).
 - FFN as 3+3 matmul slices (hidden 384 = 3x128).
 - upsample+conv pairs fused into even/odd phase k=4 convs on the
   un-upsampled grid with host-side tap-summed weights.
 - output conv k=11 tap-packed 4x into K=128 via a partition-replicated
   input built with SBUF->SBUF DMAs; sigmoid(x) = 0.5*tanh(x/2)+0.5 keeps
   ACT on the tanh/exp table.
"""

import sys

for _p in ("/opt/trn_rl_repo",):
    if _p not in sys.path:
        sys.path.insert(0, _p)

import numpy as np
from contextlib import ExitStack

import concourse.bass as bass
import concourse.bacc as bacc
import concourse.mybir as mybir
import concourse.tile as tile
from concourse.bass_utils import run_bass_kernel_spmd

F32 = mybir.dt.float32
BF16 = mybir.dt.bfloat16
AF = mybir.ActivationFunctionType
ALU = mybir.AluOpType
AX = mybir.AxisListType

L = 512
C = 96
EPS_ATTN = 1e-6
EPS_LN = 1e-5


# ----------------------------------------------------------------------------
# host-side constant prep (weight-only transforms)
# ----------------------------------------------------------------------------

def _host_prep(inp):
    f = lambda k: np.ascontiguousarray(np.asarray(inp[k], np.float32))
    p = {}
    p['Wt'] = f('Wt')                       # [96, 32] lhsT for q
    p['Wx'] = f('Wx')                       # [96, 32] lhsT for k
    Wa = f('Wa')[:, 0]
    blockWa4 = np.zeros((128, 4), np.float32)
    for c in range(4):
        blockWa4[32 * c:32 * c + 32, c] = Wa
    p['blockWa4'] = blockWa4
    p['bh4'] = np.tile(f('bh'), 4).reshape(128, 1)
    il = np.arange(128)[:, None, None]
    cc = np.arange(4)[None, :, None]
    oo = np.arange(64)[None, None, :]
    jj = cc * 128 + il + oo - 32
    p['bandmask'] = ((jj >= 0) & (jj < L)).astype(np.float32).reshape(128, 256)
    p['identity'] = np.eye(128, dtype=np.float32)
    p['identb'] = np.eye(128, dtype=np.float32)
    p['ones96'] = np.ones((96, 1), np.float32)
    p['one1_96'] = np.ones((1, 96), np.float32)
    p['one1b'] = np.ones((1, 96), np.float32)
    p['g0col'] = f('ln0_g').reshape(96, 1)
    p['b0col'] = f('ln0_b').reshape(96, 1)
    p['g1col'] = f('ln1_g').reshape(96, 1)
    p['b1col'] = f('ln1_b').reshape(96, 1)
    p['w0T'] = np.ascontiguousarray(f('ff_w0').T)                # [96, 384]
    p['fb0'] = np.ascontiguousarray(f('ff_b0').reshape(3, 128).T)  # [128, 3]
    # w1T [128, 3*96]: cols s*96+c = ff_w1[c, s*128+h]
    w1 = f('ff_w1')                                              # [96, 384]
    w1T = np.zeros((128, 288), np.float32)
    for s in range(3):
        w1T[:, s * 96:(s + 1) * 96] = w1[:, s * 128:(s + 1) * 128].T
    p['w1T'] = w1T
    p['fb1col'] = f('ff_b1').reshape(96, 1)

    def eo(w):
        # w: [co, ci, 7] -> even/odd tap-summed lhsT banks [ci, 4*co]
        We = np.stack([w[:, :, 0], w[:, :, 1] + w[:, :, 2],
                       w[:, :, 3] + w[:, :, 4], w[:, :, 5] + w[:, :, 6]])
        Wo = np.stack([w[:, :, 0] + w[:, :, 1], w[:, :, 2] + w[:, :, 3],
                       w[:, :, 4] + w[:, :, 5], w[:, :, 6]])
        co, ci = w.shape[0], w.shape[1]
        pack = lambda Ws: np.ascontiguousarray(
            Ws.transpose(2, 0, 1).reshape(ci, 4 * co))
        return pack(We), pack(Wo)

    p['W1e'], p['W1o'] = eo(f('up_w0'))   # [96, 256]
    p['W2e'], p['W2o'] = eo(f('up_w1'))   # [64, 192]
    p['W3e'], p['W3o'] = eo(f('up_w2'))   # [48, 128]

    def pack2(W, ci, co):
        # W [ci, 4*co] tap-major -> [2*ci, 2*co]: rows tau*ci+c_i,
        # group g covers taps (2g, 2g+1)
        out = np.zeros((2 * ci, 2 * co), np.float32)
        for g in range(2):
            for tau in range(2):
                t = 2 * g + tau
                out[tau * ci:(tau + 1) * ci, g * co:(g + 1) * co] = \
                    W[:, t * co:(t + 1) * co]
        return out
    p['W2e2'] = pack2(p['W2e'], 64, 48)   # [128, 96]
    p['W2o2'] = pack2(p['W2o'], 64, 48)
    p['W3e2'] = pack2(p['W3e'], 48, 32)   # [96, 64]
    p['W3o2'] = pack2(p['W3o'], 48, 32)
    p['cb1'] = f('up_b0').reshape(64, 1)
    p['cb2'] = f('up_b1').reshape(48, 1)
    p['cb3'] = f('up_b2').reshape(32, 1)
    ow = f('out_w')[0]                    # (32, 11)
    Wog = np.zeros((128, 3), np.float32)
    for g in range(3):
        for r in range(4):
            t = 4 * g + r
            if t < 11:
                Wog[32 * r:32 * r + 32, g] = ow[:, t]
    p['Wog'] = Wog
    p['ob2c'] = np.array([[f('out_b')[0] / 2.0]], np.float32)
    p['epsln'] = np.array([[EPS_LN]], np.float32)
    p['onesf'] = np.ones((96, 1), np.float32)
    p['one1f'] = np.ones((1, 96), np.float32)

    # ---- pack everything into two [128, *] blobs (2 DMAs instead of ~30) ----
    packed = {}
    for blob, names in (('wf32', F32_PACK), ('wb16', BF16_PACK)):
        width = sum(p[n].shape[1] for n in names)
        buf = np.zeros((128, width), np.float32)
        col = 0
        for n in names:
            a = p[n]
            buf[:a.shape[0], col:col + a.shape[1]] = a
            col += a.shape[1]
        packed[blob] = buf
    return {'wf32': packed['wf32'], 'wb16': packed['wb16'],
            'shapes': {n: p[n].shape for n in list(F32_PACK) + list(BF16_PACK)}}


F32_PACK = ('identity', 'bandmask', 'bh4', 'g0col', 'b0col',
            'g1col', 'b1col', 'fb1col', 'cb1', 'cb2', 'cb3', 'epsln', 'ob2c',
            'fb0', 'onesf', 'one1f')
BF16_PACK = ('Wt', 'Wx', 'blockWa4', 'ones96', 'one1b', 'identb', 'Wog',
             'w0T', 'w1T', 'W1e', 'W1o', 'W2e', 'W2o', 'W3e', 'W3o')


# ----------------------------------------------------------------------------
# device kernel build
# ----------------------------------------------------------------------------

def _bcast_free(ap_full, offset_ap, counts):
    """Custom AP on the same tensor: dims [[pstep, 128]] + counts pairs."""
    pstep = ap_full.ap[0][0]
    return bass.AP(ap_full.tensor, offset_ap.offset,
                   [[pstep, ap_full.ap[0][1]]] + list(counts))


def _build(nc, tc, t_in, t_out, tp):
    x_ap = t_in.ap()          # [96, 512]
    adense = nc.dram_tensor("adense", [4, 128, 192], BF16)  # internal scratch

    with ExitStack() as ctx:
        pw = ctx.enter_context(tc.tile_pool(name="weights", bufs=1))
        ps = ctx.enter_context(tc.tile_pool(name="seq", bufs=1))

        # Xp first: q/k depend only on it + the bf16 blob
        Xp = ps.tile([96, 576], F32, tag="Xp")
        nc.gpsimd.memset(Xp[:, 0:32], 0.0)
        nc.gpsimd.memset(Xp[:, 544:576], 0.0)
        nc.sync.dma_start(Xp[:, 32:544], x_ap)
        Xpb = ps.tile([96, 512], BF16, tag="Xpb")
        nc.vector.tensor_copy(Xpb[:], Xp[:, 32:544])

        shapes = tp['shapes']
        wf32 = pw.tile(list(tp['wf32'][1]), F32, tag="wf32")
        nc.sync.dma_start(wf32[:], tp['wf32'][0].ap())
        wb16 = pw.tile(list(tp['wb16'][1]), BF16, tag="wb16")
        nc.scalar.dma_start(wb16[:], tp['wb16'][0].ap())

        w = {}
        for blob_tile, names in ((wf32, F32_PACK), (wb16, BF16_PACK)):
            col = 0
            for n in names:
                r, cw = shapes[n]
                w[n] = blob_tile[0:r, col:col + cw]
                col += cw

        # zero adense scratch (sparsely written by the staircase DMA)
        zz = ps.tile([128, 768], BF16, tag="zz")
        nc.gpsimd.memset(zz[:], 0.0)
        nc.sync.dma_start(bass.AP(adense, 0, [[768, 128], [1, 768]]), zz[:])

        # ---------------- attention: q/k ----------------
        Q4 = ps.tile([128, 128], BF16, tag="Q4")
        K4pad = ps.tile([128, 192], BF16, tag="K4pad")
        nc.gpsimd.memset(K4pad[0:32, 0:32], 0.0)
        nc.gpsimd.memset(K4pad[96:128, 160:192], 0.0)

        with tc.tile_pool(name="qk_ps", bufs=2, space="PSUM") as pp:
            k_ps = pp.tile([128, 128], F32, tag="qk")
            for c in range(4):
                nc.tensor.matmul(k_ps[32 * c:32 * c + 32, :], w['Wx'],
                                 Xpb[:, c * 128:(c + 1) * 128],
                                 tile_position=(0, 32 * c))
            nc.scalar.activation(K4pad[:, 32:160], k_ps[:], AF.Identity,
                                 bias=w['bh4'])
            q_ps = pp.tile([128, 128], F32, tag="qk")
            for c in range(4):
                nc.tensor.matmul(q_ps[32 * c:32 * c + 32, :], w['Wt'],
                                 Xpb[:, c * 128:(c + 1) * 128],
                                 tile_position=(0, 32 * c))
            nc.scalar.copy(Q4[:], q_ps[:])
        # cross-chunk halo wings (two HWDGE queues)
        nc.sync.dma_start(K4pad[32:128, 0:32], K4pad[0:96, 128:160])
        nc.scalar.dma_start(K4pad[0:96, 160:192], K4pad[32:128, 32:64])

        # X windows for AV (depend only on Xp): transpose early while PE idle
        Xw = []
        with ExitStack() as xctx:
            xt = xctx.enter_context(tc.tile_pool(name="xw_ps", bufs=2,
                                                 space="PSUM"))
            for c in range(4):
                x_ps = xt.tile([128, 192], F32, tag="x")
                nc.tensor.transpose(x_ps[:, 0:96], Xp[:, c * 128:c * 128 + 128],
                                    w['identity'][0:96, 0:96])
                nc.tensor.transpose(x_ps[0:64, 96:192],
                                    Xp[:, c * 128 + 128:c * 128 + 192],
                                    w['identity'][0:96, 0:96])
                xw = ps.tile([128, 192], BF16, tag=f"Xw{c}")
                nc.vector.tensor_copy(xw[:, 0:96], x_ps[:, 0:96])
                nc.vector.tensor_copy(xw[0:64, 96:192], x_ps[0:64, 96:192])
                Xw.append(xw)

        # ---------------- attention: band logits ----------------
        EX = ps.tile([128, 256], F32, tag="EX")
        GO = 16  # offsets per group

        with ExitStack() as ectx:
            pa_arg = ectx.enter_context(tc.tile_pool(name="arg_sb", bufs=2))
            pa_tan = ectx.enter_context(tc.tile_pool(name="tan_sb", bufs=3))
            pe = ectx.enter_context(tc.tile_pool(name="e_ps", bufs=1, space="PSUM"))
            E_ps = pe.tile([128, 256], F32, tag="E")
            for g in range(64 // GO):
                o0 = GO * g
                Targ = pa_arg.tile([128, GO * 128], BF16, tag="Targ")
                q_b = _bcast_free(Q4[:], Q4[:], [[0, GO], [1, 128]])
                k_b = _bcast_free(K4pad[:], K4pad[:, o0:192], [[1, GO], [1, 128]])
                nc.vector.tensor_add(
                    Targ[:].rearrange("p (o i) -> p o i", o=GO), q_b, k_b)
                Ttan = pa_tan.tile([128, GO * 128], BF16, tag="Ttan")
                nc.scalar.activation(Ttan[:], Targ[:], AF.Tanh)
                for oi in range(GO):
                    o = o0 + oi
                    nc.tensor.matmul(
                        E_ps[:].rearrange("p (c o) -> p c o", o=64)[:, :, o],
                        Ttan[:, oi * 128:(oi + 1) * 128], w['blockWa4'])
            nc.scalar.activation(EX[:], E_ps[:], AF.Exp)

        nc.vector.tensor_mul(EX[:], EX[:], w['bandmask'])
        S4 = ps.tile([128, 4], F32, tag="S4")
        nc.vector.tensor_reduce(S4[:], EX[:].rearrange("p (c o) -> p c o", o=64),
                                AX.X, ALU.add)
        nc.vector.tensor_scalar_add(S4[:], S4[:], EPS_ATTN)
        R4 = ps.tile([128, 4], F32, tag="R4")
        nc.vector.reciprocal(R4[:], S4[:])
        r_b = _bcast_free(R4[:], R4[:], [[1, 4], [0, 64]])
        Abf = ps.tile([128, 256], BF16, tag="Abf")
        nc.vector.tensor_mul(Abf[:].rearrange("p (c o) -> p c o", o=64),
                             EX[:].rearrange("p (c o) -> p c o", o=64), r_b)

        # staircase scatter: adense[c][i][j'=i+o] = A[i, (c,o)]
        nc.sync.dma_start(
            bass.AP(adense, 0, [[193, 128], [128 * 192, 4], [1, 64]]),
            Abf[:].rearrange("p (c o) -> p c o", o=64))

        # ---------------- attention: AV ----------------
        vT = ps.tile([96, 512], F32, tag="vT")
        with ExitStack() as actx:
            pa = actx.enter_context(tc.tile_pool(name="av_sb", bufs=2))
            pt = actx.enter_context(tc.tile_pool(name="av_ps", bufs=2, space="PSUM"))
            pv = actx.enter_context(tc.tile_pool(name="v_ps", bufs=2, space="PSUM"))
            for c in range(4):
                Ad = pa.tile([128, 192], BF16, tag="Ad")
                eng = nc.sync if c % 2 == 0 else nc.scalar
                eng.dma_start(
                    Ad[:], bass.AP(adense, c * 128 * 192, [[192, 128], [1, 192]]))
                t_ps = pt.tile([128, 256], BF16, tag="tb")
                nc.tensor.transpose(t_ps[:, 0:128], Ad[:, 0:128], w['identb'])
                nc.tensor.transpose(t_ps[0:64, 128:256], Ad[:, 128:192],
                                    w['identb'])
                At = pa.tile([128, 256], BF16, tag="At")
                nc.vector.tensor_copy(At[:, 0:128], t_ps[:, 0:128])
                nc.vector.tensor_copy(At[0:64, 128:256], t_ps[0:64, 128:256])
                v_ps = pv.tile([96, 128], F32, tag="v")
                nc.tensor.matmul(v_ps[:], Xw[c][:, 0:96], At[:, 0:128],
                                 start=True, stop=False)
                nc.tensor.matmul(v_ps[:], Xw[c][0:64, 96:192], At[0:64, 128:256],
                                 start=False, stop=True)
                nc.vector.tensor_add(vT[:, c * 128:(c + 1) * 128], v_ps[:],
                                     Xp[:, 32 + c * 128:32 + (c + 1) * 128])

        # ---------------- LN / FFN ----------------
        def layer_norm(src_sb, gcol, bcol, out_ap):
            # var = E[x^2] - mu^2 so sum and sumsq matmuls run in parallel;
            # the centering chain (mu bcast, x - mu) overlaps the rstd chain.
            with ExitStack() as lctx:
                lp = lctx.enter_context(tc.tile_pool(name="ln_sb", bufs=1))
                lpp = lctx.enter_context(
                    tc.tile_pool(name="ln_ps", bufs=2, space="PSUM"))
                sq = lp.tile([96, 512], F32, tag="sq")
                nc.scalar.activation(sq[:], src_sb[:], AF.Square)
                s_ps = lpp.tile([1, 512], F32, tag="lns")
                nc.tensor.matmul(s_ps[:], w['onesf'], src_sb[:])
                q_ps = lpp.tile([1, 512], F32, tag="lnq")
                nc.tensor.matmul(q_ps[:], w['onesf'], sq[:])
                mub = lp.tile([1, 512], BF16, tag="mub")
                nc.scalar.mul(mub[:], s_ps[:], 1.0 / 96.0)
                mub_ps = lpp.tile([96, 512], F32, tag="lnb")
                nc.tensor.matmul(mub_ps[:], w['one1b'], mub[:])
                xc = lp.tile([96, 512], F32, tag="xc")
                nc.vector.tensor_sub(xc[:], src_sb[:], mub_ps[:])
                musq = lp.tile([1, 512], F32, tag="musq")
                nc.vector.tensor_mul(musq[:], mub[:], mub[:])
                var = lp.tile([1, 512], F32, tag="var")
                nc.vector.scalar_tensor_tensor(var[:], q_ps[:], 1.0 / 96.0,
                                               musq[:], ALU.mult, ALU.subtract)
                std = lp.tile([1, 512], F32, tag="std")
                nc.scalar.activation(std[:], var[:], AF.Sqrt,
                                     bias=w['epsln'], scale=1.0)
                rstdf = lp.tile([1, 512], F32, tag="rstdf")
                nc.vector.reciprocal_approx_fast(rstdf[:], std[:])
                rb_ps = lpp.tile([96, 512], F32, tag="lnb")
                nc.tensor.matmul(rb_ps[:], w['one1f'], rstdf[:])
                tmp = lp.tile([96, 512], F32, tag="tmp")
                nc.vector.scalar_tensor_tensor(tmp[:], xc[:], gcol[:], rb_ps[:],
                                               ALU.mult, ALU.mult)
                nc.scalar.activation(out_ap, tmp[:], AF.Identity, bias=bcol[:])

        x2 = ps.tile([96, 512], F32, tag="x2")
        layer_norm(vT, w['g0col'], w['b0col'], x2[:])
        x2b = ps.tile([96, 512], BF16, tag="x2b")
        nc.vector.tensor_copy(x2b[:], x2[:])

        x4pre = ps.tile([96, 512], F32, tag="x4pre")
        with ExitStack() as fctx:
            fp = fctx.enter_context(tc.tile_pool(name="ffn_sb", bufs=1))
            fpp = fctx.enter_context(tc.tile_pool(name="ffn_ps", bufs=1, space="PSUM"))
            Hr = []
            for s in range(3):
                h_ps = fpp.tile([128, 512], F32, tag=f"h{s}")
                nc.tensor.matmul(h_ps[:], w['w0T'][:, s * 128:(s + 1) * 128], x2b[:])
                hr = fp.tile([128, 512], BF16, tag=f"hr{s}")
                nc.scalar.activation(hr[:], h_ps[:], AF.Relu,
                                     bias=w['fb0'][:, s:s + 1])
                Hr.append(hr)
            x3_ps = fpp.tile([96, 512], F32, tag="x3")
            for s in range(3):
                nc.tensor.matmul(x3_ps[:], w['w1T'][:, s * 96:(s + 1) * 96], Hr[s][:],
                                 start=(s == 0), stop=(s == 2))
            nc.vector.scalar_tensor_tensor(x4pre[:], x3_ps[:], w['fb1col'],
                                           x2[:], ALU.add, ALU.add)

        h0 = ps.tile([96, 516], BF16, tag="h0")
        nc.gpsimd.memset(h0[:, 0:2], 0.0)
        nc.gpsimd.memset(h0[:, 514:516], 0.0)
        layer_norm(x4pre, w['g1col'], w['b1col'], h0[:, 2:514])

        # ---------------- conv stack ----------------
        def conv_eo(hin, We, Wo, bcol, cin, cout, G, hout, hout_pad):
            nchunk = G // 512
            with ExitStack() as cctx:
                cp = cctx.enter_context(
                    tc.tile_pool(name=f"c{cout}_ps", bufs=4, space="PSUM"))
                for k in range(nchunk):
                    for par, Wp in ((0, We), (1, Wo)):
                        z_ps = cp.tile([cout, 512], F32, tag="z")
                        for tau in range(4):
                            rhs = hin[:, par + tau + k * 512:
                                      par + tau + k * 512 + 512]
                            nc.tensor.matmul(z_ps[:],
                                             Wp[:, tau * cout:(tau + 1) * cout],
                                             rhs, start=(tau == 0), stop=(tau == 3))
                        col0 = hout_pad + 2 * k * 512 + par
                        dst = bass.AP(hout[:].tensor, hout[:, col0:col0 + 1].offset,
                                      [[hout[:].ap[0][0], cout], [2, 512]])
                        if par == 0:
                            nc.scalar.activation(dst, z_ps[:], AF.Relu,
                                                 bias=bcol[:])
                        else:
                            nc.vector.tensor_scalar(dst, z_ps[:], bcol[:], 0.0,
                                                    ALU.add, ALU.max)

        h1 = ps.tile([64, 1028], BF16, tag="h1")
        nc.gpsimd.memset(h1[:, 0:2], 0.0)
        nc.gpsimd.memset(h1[:, 1026:1028], 0.0)
        conv_eo(h0, w['W1e'], w['W1o'], w['cb1'], 96, 64, 512, h1, 2)

        h2 = ps.tile([48, 2052], BF16, tag="h2")
        nc.gpsimd.memset(h2[:, 0:2], 0.0)
        nc.gpsimd.memset(h2[:, 2050:2052], 0.0)
        conv_eo(h1, w['W2e'], w['W2o'], w['cb2'], 64, 48, 1024, h2, 2)

        h3rep = ps.tile([128, 4112], BF16, tag="h3rep")
        nc.gpsimd.memset(h3rep[0:32, 0:8], 0.0)
        nc.gpsimd.memset(h3rep[:, 4104:4112], 0.0)
        conv_eo(h2, w['W3e'], w['W3o'], w['cb3'], 48, 32, 2048, h3rep, 8)
        for r in range(1, 4):
            nc.sync.dma_start(h3rep[32 * r:32 * r + 32, 0:4112 - r],
                              h3rep[0:32, r:4112])

        # ---------------- output conv + sigmoid ----------------
        ysig = ps.tile([1, 4096], F32, tag="ysig")
        out_sb = ps.tile([1, 4096], F32, tag="out_sb")
        with tc.tile_pool(name="yo_ps", bufs=2, space="PSUM") as yp:
            for k in range(8):
                y_ps = yp.tile([1, 512], F32, tag="yo")
                for g in range(3):
                    rhs = h3rep[:, k * 512 + 4 * g + 3: k * 512 + 4 * g + 3 + 512]
                    nc.tensor.matmul(y_ps[:], w['Wog'][:, g:g + 1], rhs,
                                     start=(g == 0), stop=(g == 2))
                nc.scalar.activation(ysig[:, k * 512:(k + 1) * 512], y_ps[:],
                                     AF.Tanh, bias=w['ob2c'], scale=0.5)
                nc.vector.tensor_scalar(out_sb[:, k * 512:(k + 1) * 512],
                                        ysig[:, k * 512:(k + 1) * 512],
                                        0.5, 0.5, ALU.mult, ALU.add)
        nc.sync.dma_start(t_out.ap(), out_sb[:])


# ----------------------------------------------------------------------------
# public entry point
# ----------------------------------------------------------------------------

def build_module(p, repeat=1):
    nc = bacc.Bacc("TRN2", target_bir_lowering=False, debug=False)
    t_in = nc.dram_tensor("x", [C, L], F32, kind="ExternalInput")
    t_out = nc.dram_tensor("out", [1, 4096], F32, kind="ExternalOutput")
    tp = {
        'wf32': (nc.dram_tensor('wf32', list(p['wf32'].shape), F32,
                                kind="ExternalInput"), p['wf32'].shape, F32),
        'wb16': (nc.dram_tensor('wb16', list(p['wb16'].shape), BF16,
                                kind="ExternalInput"), p['wb16'].shape, BF16),
    }
    tp['shapes'] = p['shapes']
    with tile.TileContext(nc) as tc:
        if repeat == 1:
            _build(nc, tc, t_in, t_out, tp)
        else:
            engs = [mybir.EngineType.PE, mybir.EngineType.DVE,
                    mybir.EngineType.Activation, mybir.EngineType.SP,
                    mybir.EngineType.Pool]
            with tc.For_i(0, repeat, 1, hint_engines=engs):
                _build(nc, tc, t_in, t_out, tp)
    nc.compile()
    return nc


def kernel(**inputs):
    # The neuron compile cache keys on the HLO signature only (it does not
    # hash the embedded bass program), so a stale entry from a different
    # kernel revision with identical I/O shapes would silently load the
    # wrong NEFF. Purge before compiling.
    import shutil
    shutil.rmtree('/root/.neuron-compile-cache', ignore_errors=True)
    shutil.rmtree('/var/tmp/neuron-compile-cache', ignore_errors=True)

    x = np.asarray(inputs['x'], np.float32)          # (8, 96, 512)
    N = x.shape[0]
    p = _host_prep(inputs)
    nc = build_module(p)
    import ml_dtypes
    feed = {'wf32': np.ascontiguousarray(p['wf32'], np.float32),
            'wb16': np.ascontiguousarray(p['wb16'], ml_dtypes.bfloat16)}
    in_maps = []
    for n in range(N):
        m = dict(feed)
        m['x'] = np.ascontiguousarray(x[n])
        in_maps.append(m)
    res = run_bass_kernel_spmd(nc, in_maps, core_ids=list(range(N)))
    global LAST_RESULTS
    LAST_RESULTS = res
    out = np.stack([res.results[n]['out'] for n in range(N)], axis=0)
    return out.astype(np.float32)


LAST_RESULTS = None


if __name__ == '__main__':
    print("kernel.py loaded OK")
